# revision 15
# baseline (speedup 1.0000x reference)
"""Trainium2 Bass kernel for GAT-style attention softmax (CochainMessagePassing).

Computes, for inputs
    x       [4, 4, 1024, 512]  f32
    attn_w  [4, 4, 8, 1024, 128] f32
the output
    out     [4, 4, 1024, 8, 1024] f32
where per (b, n, head h):
    xh   = x[b, n, :, h*64:(h+1)*64]            # [1024, 64]
    a2   = attn_w[b, n, h, :, 64:128]           # [1024, 64]
    e    = a2 @ xh.T                            # [1024, 1024]
    out[b, n, i, h, j] = softmax_j(e_self[i] + e[i, j]) = softmax_j(e[i, j])
(e_self is constant along the softmax axis so it cancels; a1 is never needed).

Sharding: the 16 (b, n) slabs are split 2-per-core across 8 NeuronCores
(pure data parallel, no collectives).

v5 pipeline per slab:
  - SWDGE DMA loads cast f32 -> fp16 in flight (free dtype conversion)
  - x / a2 transposed by ONE SBUF->SBUF xbar DMA each (dma_start_transpose,
    2-byte dtype) -- no PE transposes, no PSUM traffic, PSUM is scores-only
  - scores: fp16 matmuls (K=64, alternating row groups h0/h64), f32 PSUM
    [128, 2048] double-buffered pair tiles (all 8 banks)
  - exp: hybrid -- half the pairs run per-head ACT exp with accum_out row
    sums (a sum costs ~0.3us there); the rest run pair-batched [128, 2048]
    ACT exp (cheaper per element) with DVE cache-reduce sums (~1.2us each).
    The split balances ACT vs DVE busy time.
  - pair-batched reciprocals; per-head normalize multiply exp_sb -> out_sb
  - one 2 MB DMA per (slab, i-block): out_sb [128, 8*1024] bf16 -> HBM
Output is stored bf16 on device and upcast to f32 on the host.
fp16 matmul inputs + bf16 output keep rel err ~6e-3 (tolerance 2e-2).
"""

import sys

sys.path.insert(0, "/opt/trn_rl_repo")

from contextlib import ExitStack

import numpy as np

import concourse.bass as bass
import concourse.tile as tile
from concourse import mybir
from concourse.bass_utils import run_bass_kernel_spmd

NUM_CORES = 8
SLABS_PER_CORE = 2  # (b, n) pairs per core
N_C = 1024  # complexes
D = 512
H = 8  # heads
DH = 64  # head dim
NIB = N_C // 128  # i-blocks per slab

F32 = mybir.dt.float32
FP16 = mybir.dt.float16
BF16 = mybir.dt.bfloat16

MM_DT = FP16
OUT_DT = BF16


def make_pools(ctx: ExitStack, tc: tile.TileContext):
    nc = tc.nc
    pools = {}
    pools["const"] = ctx.enter_context(tc.tile_pool(name="const", bufs=1))
    pools["xstage"] = ctx.enter_context(tc.tile_pool(name="xstage", bufs=2))
    pools["xT"] = ctx.enter_context(tc.tile_pool(name="xT", bufs=2))
    pools["a2stage"] = ctx.enter_context(tc.tile_pool(name="a2stage", bufs=2))
    pools["a2T"] = ctx.enter_context(tc.tile_pool(name="a2T", bufs=2))
    pools["outp"] = ctx.enter_context(tc.tile_pool(name="outp", bufs=3))
    pools["expp"] = ctx.enter_context(tc.tile_pool(name="expp", bufs=4))
    pools["stat"] = ctx.enter_context(tc.tile_pool(name="stat", bufs=8))
    # single PSUM pool: [128, 2048] f32 tiles = 4 banks each, 2 bufs = all 8
    pools["psum"] = ctx.enter_context(tc.tile_pool(name="psum", bufs=2, space="PSUM"))
    return pools


def build_kernel_body(pools, tc: tile.TileContext, out_ap, x_ap, w_ap):
    nc = tc.nc
    xstage = pools["xstage"]
    xT_pool = pools["xT"]
    a2stage = pools["a2stage"]
    a2T_pool = pools["a2T"]
    outp = pools["outp"]
    expp = pools["expp"]
    stat_pool = pools["stat"]
    pspool = pools["psum"]

    for s in range(SLABS_PER_CORE):
        # ---- stage x[s]: one casting DMA into fp16 [128, 8*512] (jb-blocked) ----
        # x_sb[p, jb*512 + d] = fp16(x[s, jb*128 + p, d])
        x_sb = xstage.tile([128, NIB * D], MM_DT)
        nc.gpsimd.dma_start(
            x_sb[:].rearrange("p (a d) -> p a d", a=NIB),
            x_ap[s].rearrange("(a p) d -> p a d", p=128),
        )

        # ---- transpose x[s] via SBUF->SBUF xbar DMA ----
        # xT[dd, jb*512 + p*128 + j] = x_sb[j, jb*512 + p*128 + dd]
        # (xbar full transpose of [128, 4096] -> out AP [128, 32, 128])
        xT = xT_pool.tile([128, 4 * N_C], MM_DT)
        nc.sync.dma_start_transpose(
            xT[:].rearrange("dd (b j) -> dd b j", j=128), x_sb[:]
        )
        xT_b = xT[:].rearrange("dd (jb p j) -> dd jb p j", p=4, j=128)

        # ---- stage a2 (casting DMA) + xbar transpose ----
        # a2s[p, q*N_C + ib*128 + hh*64 + k] = fp16(w[s, 2q+hh, ib*128+p, 64+k])
        # a2T[hh*64+k, q*N_C + ib*128 + i] = a2 of head 2q+hh at [ib*128+i, k]
        a2s = a2stage.tile([128, 4 * N_C], MM_DT)
        a2T = a2T_pool.tile([128, 4 * N_C], MM_DT)
        for q in range(4):
            a2s_r = a2s[:, q * N_C : (q + 1) * N_C].rearrange("p (a c) -> p a c", c=128)
            for hh in range(2):
                h = 2 * q + hh
                src = w_ap[s, h, :, DH : 2 * DH].rearrange("(a p) k -> p a k", p=128)
                nc.gpsimd.dma_start(a2s_r[:, :, hh * DH : (hh + 1) * DH], src)
        nc.sync.dma_start_transpose(
            a2T[:].rearrange("k (b i) -> k b i", i=128), a2s[:]
        )

        # ---- scores + softmax, i-block major; one output DMA per i-block ----
        # Sums are the contended resource: on ACT (accum_out during a
        # per-head exp) a sum costs ~0.27us extra; on DVE (tensor_scalar
        # cache-reduce over the exp tile) ~1.2us. Pair-batched ACT exp
        # ([128,2048]) is ~0.5us/pair cheaper than 2 per-head exps but
        # forfeits accum_out (it would mix the two heads). Split pairs
        # between the two modes to balance ACT vs DVE load.
        for ib in range(NIB):
            out_sb = outp.tile([128, H * N_C], OUT_DT)
            sums = stat_pool.tile([128, H], F32, tag="sums")
            recs = stat_pool.tile([128, H], F32, tag="recs")
            for g in range(4):  # head pair (2g, 2g+1)
                psc = pspool.tile([128, 2048], F32, tag="ps")
                for hh in range(2):
                    lhsT = a2T[
                        hh * DH : (hh + 1) * DH,
                        g * N_C + ib * 128 : g * N_C + (ib + 1) * 128,
                    ]
                    for jc in range(2):
                        rhs = xT_b[
                            hh * DH : (hh + 1) * DH, jc * 4 : (jc + 1) * 4, g, :
                        ]
                        nc.tensor.matmul(
                            psc[
                                :,
                                hh * N_C + jc * 512 : hh * N_C + (jc + 1) * 512,
                            ],
                            lhsT,
                            rhs,
                            start=True,
                            stop=True,
                        )
                exp_sb = expp.tile([128, 2 * N_C], OUT_DT)
                # ~ν of pairs: per-head ACT exp with free-ish accum sums
                act_sums = g < 2
                if act_sums:
                    for hh in range(2):
                        h = 2 * g + hh
                        nc.scalar.activation(
                            exp_sb[:, hh * N_C : (hh + 1) * N_C],
                            psc[:, hh * N_C : (hh + 1) * N_C],
                            mybir.ActivationFunctionType.Exp,
                            accum_out=sums[:, h : h + 1],
                        )
                else:
                    nc.scalar.activation(
                        exp_sb[:], psc[:], mybir.ActivationFunctionType.Exp
                    )
                    for hh in range(2):
                        h = 2 * g + hh
                        sl = exp_sb[:, hh * N_C : (hh + 1) * N_C]
                        nc.vector.tensor_scalar(
                            sl,
                            sl,
                            1.0,
                            None,
                            op0=mybir.AluOpType.mult,
                            op1=mybir.AluOpType.add,
                            accum_out=sums[:, h : h + 1],
                        )
                nc.vector.reciprocal(
                    recs[:, 2 * g : 2 * g + 2], sums[:, 2 * g : 2 * g + 2]
                )
                for hh in range(2):
                    h = 2 * g + hh
                    nc.vector.tensor_scalar_mul(
                        out_sb[:, h * N_C : (h + 1) * N_C],
                        exp_sb[:, hh * N_C : (hh + 1) * N_C],
                        recs[:, h : h + 1],
                    )
            nc.sync.dma_start(
                out_ap[s, ib * 128 : (ib + 1) * 128, :, :],
                out_sb[:].rearrange("p (h j) -> p h j", h=H),
            )


def _split_multi_waits(nc):
    """walrus's per-instruction codegen structs hold only one embedded sync
    wait; hoist multi-wait instructions' waits onto standalone same-engine
    wait instructions placed immediately before them (program order on the
    sequencer preserves semantics)."""
    ctr = 0
    for f in nc.m.functions:
        for blk in f.blocks:
            out = []
            changed = False
            for inst in blk.instructions:
                tname = type(inst).__name__
                si = inst.sync_info
                if (
                    tname != "InstEventSemaphore"
                    and si is not None
                    and si.on_wait
                    and len(si.on_wait) > 1
                ):
                    for w in si.on_wait:
                        wi = mybir.InstEventSemaphore(name=f"WSPLIT-{ctr}")
                        ctr += 1
                        wi.engine = inst.engine
                        wi.sync_info = mybir.SyncInfo(on_wait=[w], on_update=[])
                        out.append(wi)
                    inst.sync_info = mybir.SyncInfo(
                        on_wait=[], on_update=list(si.on_update)
                    )
                    changed = True
                out.append(inst)
            if changed:
                blk.instructions = out
    return ctr


def build_bass(bench_repeats=None, split_waits=True):
    nc = bass.Bass("TRN2", target_bir_lowering=False, debug=False)
    if bench_repeats is None:
        x_ap = nc.dram_tensor(
            "x", [SLABS_PER_CORE, N_C, D], F32, kind="ExternalInput"
        ).ap()
        w_ap = nc.dram_tensor(
            "attn_w", [SLABS_PER_CORE, H, N_C, 2 * DH], F32, kind="ExternalInput"
        ).ap()
        out_ap = nc.dram_tensor(
            "out", [SLABS_PER_CORE, N_C, H, N_C], OUT_DT, kind="ExternalOutput"
        ).ap()
        with tile.TileContext(nc) as tc:
            with ExitStack() as ctx:
                pools = make_pools(ctx, tc)
                build_kernel_body(pools, tc, out_ap, x_ap, w_ap)
    else:
        # bench variant: all big tensors are device-internal (no host I/O);
        # tiny external in/out keep the custom-call ABI happy.
        x_ap = nc.dram_tensor("xi", [SLABS_PER_CORE, N_C, D], F32).ap()
        w_ap = nc.dram_tensor("wi", [SLABS_PER_CORE, H, N_C, 2 * DH], F32).ap()
        out_ap = nc.dram_tensor("oi", [SLABS_PER_CORE, N_C, H, N_C], OUT_DT).ap()
        tin = nc.dram_tensor("tin", [1, 4], F32, kind="ExternalInput").ap()
        tout = nc.dram_tensor("tout", [1, 4], F32, kind="ExternalOutput").ap()
        with tile.TileContext(nc) as tc:
            with ExitStack() as ctx:
                pools = make_pools(ctx, tc)
                tiny = pools["const"].tile([1, 4], F32)
                nc.gpsimd.dma_start(tiny[:], tin[:, :])
                nc.gpsimd.dma_start(tout[:, :], tiny[:])
                zt = pools["const"].tile([128, 4 * N_C], F32)
                nc.vector.memset(zt[:], 0.0)
                x_flat = x_ap.rearrange("s (a p) d -> (s a) p d", p=128)
                for t in range(x_flat.shape[0]):
                    nc.gpsimd.dma_start(x_flat[t], zt[:, :D])
                w_flat = w_ap.rearrange("s h (a p) k -> (s h a) p k", p=128)
                for t in range(w_flat.shape[0]):
                    nc.gpsimd.dma_start(w_flat[t], zt[:, : 2 * DH])
                for _ in range(bench_repeats):
                    build_kernel_body(pools, tc, out_ap, x_ap, w_ap)
    if split_waits:
        _split_multi_waits(nc)
    return nc


_NC_CACHE = None


def _get_nc():
    global _NC_CACHE
    if _NC_CACHE is None:
        _NC_CACHE = build_bass()
    return _NC_CACHE


def kernel(x: np.ndarray, attn_w: np.ndarray, _trace: bool = False):
    assert x.shape == (4, 4, N_C, D), x.shape
    assert attn_w.shape == (4, 4, H, N_C, 2 * DH), attn_w.shape
    xs = np.ascontiguousarray(x, dtype=np.float32).reshape(16, N_C, D)
    ws = np.ascontiguousarray(attn_w, dtype=np.float32).reshape(16, H, N_C, 2 * DH)
    in_maps = [
        {
            "x": np.ascontiguousarray(xs[2 * c : 2 * c + 2]),
            "attn_w": np.ascontiguousarray(ws[2 * c : 2 * c + 2]),
        }
        for c in range(NUM_CORES)
    ]
    nc = _get_nc()
    res = run_bass_kernel_spmd(
        nc, in_maps, core_ids=list(range(NUM_CORES)), trace=_trace
    )
    out = np.concatenate([res.results[c]["out"] for c in range(NUM_CORES)], axis=0)
    if _trace:
        kernel.last_exec_time_ns = res.exec_time_ns
        it = res.instructions_and_trace
        kernel.last_trace_path = it[1] if it else None
    return out.reshape(4, 4, N_C, H, N_C).astype(np.float32)


kernel.last_exec_time_ns = None
kernel.last_trace_path = None
